# revision 8
# baseline (speedup 1.0000x reference)
"""Multi-head attention (B=2, S=2048, D=1024, H=16) on 8 TRN2 NeuronCores.

Sharding: DP=2 over batch x TP=4 over heads (4 heads/core).
Per core: QKV projections for its 256 output dims, attention for its 4
heads on its batch, row-parallel output projection producing a partial
[2048, 1024]; host sums the 4 partials per batch and adds bo.

Layout strategy (per core):
  - host pre-transposes x (q/k/v) to xT [1024, 2048] and weight slices
  - q/k projections produce qhT/khT [o, s] (o on partitions, stacked in
    head-pairs of 128); v projection produces vh natural [s, o]
  - scores computed transposed, sT[j, i] = khT.T @ qhT, two heads packed
    on the 128x128 PE array via row tiling (K=64 each)
  - softmax: exp on ACT (scale=1/8 folded in, no max subtraction needed
    since scores ~ N(0,1)); denominator via an appended ones-column in
    the attnV matmul (lhsT = [vh | 1] -> out rows 0..63 = outT, row 64 =
    rowsum); normalization folded in before the out projection
  - all matmul operands float32r (full PE rate, ~1.9e-4 matmul rel err)
"""
import numpy as np

B, S, D = 2, 2048, 1024
HEADS, DK = 16, 64
NCORES, DP, TP = 8, 2, 4
OPC = D // TP          # 256 output dims per core
HPC = HEADS // TP      # 4 heads per core
NDC = D // 128         # 8 contraction chunks
NST = S // 128         # 16 s-tiles
NSC = S // 512         # 4 s-chunks

_cache = {}


def _build():
    import concourse.mybir as mybir
    import concourse.tile as tile
    from concourse import bacc

    F32 = mybir.dt.float32
    F32R = mybir.dt.float32r
    Exp = mybir.ActivationFunctionType.Exp
    Copy = mybir.ActivationFunctionType.Copy
    Ident = mybir.ActivationFunctionType.Identity

    nc = bacc.Bacc("TRN2", target_bir_lowering=False, debug=False)

    xq_d = nc.dram_tensor("xqt", [D, S], F32R, kind="ExternalInput")
    xk_d = nc.dram_tensor("xkt", [D, S], F32R, kind="ExternalInput")
    xv_d = nc.dram_tensor("xvt", [D, S], F32R, kind="ExternalInput")
    wq_d = nc.dram_tensor("wqt", [D, OPC], F32R, kind="ExternalInput")
    wk_d = nc.dram_tensor("wkt", [D, OPC], F32R, kind="ExternalInput")
    wv_d = nc.dram_tensor("wvt", [D, OPC], F32R, kind="ExternalInput")
    bq_d = nc.dram_tensor("bq", [2, 128, 1], F32, kind="ExternalInput")
    bk_d = nc.dram_tensor("bk", [2, 128, 1], F32, kind="ExternalInput")
    bv_d = nc.dram_tensor("bv", [1, OPC], F32R, kind="ExternalInput")
    wo_d = nc.dram_tensor("wot", [2, 128, D], F32R, kind="ExternalInput")
    out_d = nc.dram_tensor("out", [S, D], F32, kind="ExternalOutput")

    with tile.TileContext(nc) as tc:
        from contextlib import ExitStack
        es = ExitStack()
        with es:
            wp = es.enter_context(tc.tile_pool(name="wp", bufs=1))
            acts = es.enter_context(tc.tile_pool(name="acts", bufs=1))

            # ---- phase 0: weights, biases, constants
            wq_t = [wp.tile([128, OPC], F32R, name=f"wq{i}") for i in range(NDC)]
            wk_t = [wp.tile([128, OPC], F32R, name=f"wk{i}") for i in range(NDC)]
            wv_t = [wp.tile([128, OPC], F32R, name=f"wv{i}") for i in range(NDC)]
            for i in range(NDC):
                nc.scalar.dma_start(wq_t[i][:], wq_d.ap()[i * 128:(i + 1) * 128, :])
                nc.scalar.dma_start(wk_t[i][:], wk_d.ap()[i * 128:(i + 1) * 128, :])
                nc.scalar.dma_start(wv_t[i][:], wv_d.ap()[i * 128:(i + 1) * 128, :])
            wo_t = [wp.tile([128, D], F32R, name=f"wo{h}") for h in range(2)]
            bq_t = [wp.tile([128, 1], F32, name=f"bq{h}") for h in range(2)]
            bk_t = [wp.tile([128, 1], F32, name=f"bk{h}") for h in range(2)]
            for h in range(2):
                nc.sync.dma_start(wo_t[h][:], wo_d.ap()[h])
                nc.sync.dma_start(bq_t[h][:], bq_d.ap()[h])
                nc.sync.dma_start(bk_t[h][:], bk_d.ap()[h])
            bv_r = wp.tile([1, OPC], F32R, name="bv_r")
            nc.sync.dma_start(bv_r[:], bv_d.ap())

            ones_f = wp.tile([128, 1], F32, name="ones_f")
            nc.vector.memset(ones_f[:], 1.0)
            ones1 = wp.tile([1, 128], F32R, name="ones1")
            onesf1 = wp.tile([1, 128], F32, name="onesf1")
            nc.vector.memset(onesf1[:], 1.0)
            nc.vector.tensor_copy(ones1[:], onesf1[:])

            # bv broadcast to all 128 partitions: bv2 = ones1.T @ bv_r
            with tc.tile_pool(name="bv_ps", bufs=1, space="PSUM") as bvps:
                pbv = bvps.tile([128, OPC], F32, name="pbv")
                nc.tensor.matmul(pbv[:], ones1[:], bv_r[:], start=True, stop=True)
                bv2 = wp.tile([128, OPC], F32, name="bv2")
                nc.vector.tensor_copy(bv2[:], pbv[:])

            # persistent activations
            qh_st = [acts.tile([128, S], F32R, name=f"qh{h}") for h in range(2)]
            kh_st = [acts.tile([128, S], F32R, name=f"kh{h}") for h in range(2)]
            vh_aug = [[acts.tile([128, DK + 1], F32R, name=f"va{h}_{j}")
                       for j in range(NST)] for h in range(HPC)]
            stacked = [acts.tile([128, S], F32R, name=f"st{h}") for h in range(2)]

            # ---- phase 1: projections
            with tc.tile_pool(name="xin", bufs=1) as xp, \
                 tc.tile_pool(name="pps", bufs=4, space="PSUM") as pps, \
                 tc.tile_pool(name="vps", bufs=4, space="PSUM") as vps:
                for xd, wt, dest in ((xq_d, wq_t, qh_st), (xk_d, wk_t, kh_st)):
                    bias = bq_t if dest is qh_st else bk_t
                    for sc in range(NSC):
                        xt = [xp.tile([128, 512], F32R, name="xt", tag="xt", bufs=18)
                              for _ in range(NDC)]
                        for dc in range(NDC):
                            eng = nc.sync if dc % 2 == 0 else nc.scalar
                            eng.dma_start(
                                xt[dc][:], xd.ap()[dc * 128:(dc + 1) * 128,
                                                   sc * 512:(sc + 1) * 512])
                        for hp in range(2):
                            p = pps.tile([128, 512], F32, name="pp", tag="pp")
                            for dc in range(NDC):
                                nc.tensor.matmul(
                                    p[:], wt[dc][:, hp * 128:(hp + 1) * 128],
                                    xt[dc][:], start=(dc == 0), stop=(dc == NDC - 1))
                            nc.scalar.activation(
                                dest[hp][:, sc * 512:(sc + 1) * 512], p[:],
                                Ident, bias=bias[hp][:])
                # v projection: natural layout, x as stationary
                for sc in range(NSC):
                    xt = [xp.tile([128, 512], F32R, name="xt", tag="xt", bufs=18)
                          for _ in range(NDC)]
                    for dc in range(NDC):
                        eng = nc.sync if dc % 2 == 0 else nc.scalar
                        eng.dma_start(
                            xt[dc][:], xv_d.ap()[dc * 128:(dc + 1) * 128,
                                                 sc * 512:(sc + 1) * 512])
                    for st4 in range(4):
                        st = sc * 4 + st4
                        pv = vps.tile([128, OPC], F32, name="pv", tag="pv")
                        for dc in range(NDC):
                            nc.tensor.matmul(
                                pv[:], xt[dc][:, st4 * 128:(st4 + 1) * 128],
                                wv_t[dc][:], start=(dc == 0), stop=(dc == NDC - 1))
                        for h in range(HPC):
                            nc.vector.tensor_add(
                                vh_aug[h][st][:, 0:DK], pv[:, h * DK:(h + 1) * DK],
                                bv2[:, h * DK:(h + 1) * DK])
                            nc.vector.tensor_copy(vh_aug[h][st][:, DK:DK + 1],
                                                  ones_f[:])

            # ---- PE warmup: ~4us of back-to-back matmuls to trip HAM to 8/8
            with tc.tile_pool(name="wups", bufs=1, space="PSUM") as wups:
                wub = wups.tile([128, 512], F32, name="wub")
                for wi in range(18):
                    nc.tensor.matmul(wub[:], kh_st[0][:, 0:128],
                                     qh_st[0][:, 0:512],
                                     start=(wi == 0), stop=(wi == 17))

            # ---- phase 2: attention per head-pair
            with tc.tile_pool(name="sps", bufs=2, space="PSUM") as sps, \
                 tc.tile_pool(name="avps", bufs=3, space="PSUM") as avps, \
                 tc.tile_pool(name="r2ps", bufs=1, space="PSUM") as r2ps, \
                 tc.tile_pool(name="ep", bufs=4) as ep, \
                 tc.tile_pool(name="rp", bufs=4) as rp:
                for hp in range(2):
                    for ic in range(NSC):
                        av = [avps.tile([128, 512], F32, name="av", tag="av")
                              for _ in range(2)]
                        for j in range(NST):
                            sp = sps.tile([128, 1024], F32, name="sp", tag="sp")
                            nc.tensor.matmul(
                                sp[:, 0:512], kh_st[hp][0:64, j * 128:(j + 1) * 128],
                                qh_st[hp][0:64, ic * 512:(ic + 1) * 512],
                                start=True, stop=True, tile_position=(0, 0))
                            nc.tensor.matmul(
                                sp[:, 512:1024], kh_st[hp][64:128, j * 128:(j + 1) * 128],
                                qh_st[hp][64:128, ic * 512:(ic + 1) * 512],
                                start=True, stop=True, tile_position=(64, 0))
                            et = ep.tile([128, 1024], F32R, name="et", tag="et")
                            nc.scalar.activation(et[:], sp[:], Exp, scale=0.125)
                            for h2 in range(2):
                                nc.tensor.matmul(
                                    av[h2][0:DK + 1, :], vh_aug[hp * 2 + h2][j][:],
                                    et[:, h2 * 512:(h2 + 1) * 512],
                                    start=(j == 0), stop=(j == NST - 1),
                                    skip_group_check=True)
                        for h2 in range(2):
                            dnm = rp.tile([1, 512], F32, name="dnm", tag="dnm")
                            nc.vector.tensor_copy(dnm[:], av[h2][DK:DK + 1, :])
                            rcf = rp.tile([1, 512], F32, name="rcf", tag="rcf")
                            nc.vector.reciprocal_approx_fast(rcf[:], dnm[:])
                            rci = rp.tile([1, 512], F32R, name="rci", tag="rci")
                            nc.vector.tensor_copy(rci[:], rcf[:])
                            r2 = r2ps.tile([64, 512], F32, name="r2", tag="r2")
                            nc.tensor.matmul(r2[:], ones1[0:1, 0:64], rci[:],
                                             start=True, stop=True)
                            r2s = rp.tile([64, 512], F32, name="r2s", tag="r2s")
                            nc.scalar.activation(r2s[:], r2[:], Copy)
                            nc.vector.tensor_mul(
                                stacked[hp][h2 * 64:(h2 + 1) * 64,
                                            ic * 512:(ic + 1) * 512],
                                av[h2][0:DK, :], r2s[:])

            # ---- phase 3: output projection (partial; host adds across cores)
            with tc.tile_pool(name="ops", bufs=4, space="PSUM") as ops, \
                 tc.tile_pool(name="obp", bufs=4) as obp:
                for it in range(NST):
                    for mc in range(2):
                        po = ops.tile([128, 512], F32, name="po", tag="po")
                        for hp in range(2):
                            nc.tensor.matmul(
                                po[:], stacked[hp][:, it * 128:(it + 1) * 128],
                                wo_t[hp][:, mc * 512:(mc + 1) * 512],
                                start=(hp == 0), stop=(hp == 1))
                        ot = obp.tile([128, 512], F32, name="ot", tag="ot")
                        nc.vector.tensor_copy(ot[:], po[:])
                        nc.sync.dma_start(
                            out_d.ap()[it * 128:(it + 1) * 128,
                                       mc * 512:(mc + 1) * 512], ot[:])

    nc.compile()
    return nc


def _prep_inputs(q, k, v, Wq, bq, Wk, bk, Wv, bv, Wo, bo):
    f = np.float32
    xT = {}
    for g in range(DP):
        xT[("q", g)] = np.ascontiguousarray(np.asarray(q[g], f).T)
        xT[("k", g)] = np.ascontiguousarray(np.asarray(k[g], f).T)
        xT[("v", g)] = np.ascontiguousarray(np.asarray(v[g], f).T)
    Wq, Wk, Wv, Wo = (np.asarray(a, f) for a in (Wq, Wk, Wv, Wo))
    bq, bk, bv = (np.asarray(a, f) for a in (bq, bk, bv))
    in_maps = []
    for c in range(NCORES):
        g, r = divmod(c, TP)
        sl = slice(r * OPC, (r + 1) * OPC)
        in_maps.append({
            "xqt": xT[("q", g)], "xkt": xT[("k", g)], "xvt": xT[("v", g)],
            "wqt": np.ascontiguousarray(Wq[sl].T),
            "wkt": np.ascontiguousarray(Wk[sl].T),
            "wvt": np.ascontiguousarray(Wv[sl].T),
            "bq": bq[sl].reshape(2, 128, 1),
            "bk": bk[sl].reshape(2, 128, 1),
            "bv": bv[sl].reshape(1, OPC),
            "wot": np.ascontiguousarray(Wo[:, sl].T).reshape(2, 128, D),
        })
    return in_maps


def kernel(q, k, v, Wq, bq, Wk, bk, Wv, bv, Wo, bo, _trace=False):
    from concourse.bass_utils import run_bass_kernel_spmd

    if "nc" not in _cache:
        _cache["nc"] = _build()
    nc = _cache["nc"]
    in_maps = _prep_inputs(q, k, v, Wq, bq, Wk, bk, Wv, bv, Wo, bo)
    res = run_bass_kernel_spmd(nc, in_maps, list(range(NCORES)), trace=_trace)
    _cache["last_exec_time_ns"] = res.exec_time_ns
    _cache["last_res"] = res
    parts = [res.results[c]["out"] for c in range(NCORES)]
    bo = np.asarray(bo, np.float32)
    out = np.empty((B, S, D), np.float32)
    for g in range(DP):
        acc = parts[g * TP].astype(np.float32)
        for r in range(1, TP):
            acc = acc + parts[g * TP + r]
        out[g] = acc + bo
    return out


# revision 10
# speedup vs baseline: 1.1421x; 1.1421x over previous
"""Multi-head attention (B=2, S=2048, D=1024, H=16) on 8 TRN2 NeuronCores.

Sharding: DP=2 over batch x TP=4 over heads (4 heads/core).
Per core: QKV projections for its 256 output dims, attention for its 4
heads on its batch, row-parallel output projection producing a partial
[2048, 1024]; host sums the 4 partials per batch and adds bo.

Layout strategy (per core):
  - host pre-transposes x (q/k/v) to xT [1024, 2048] and weight slices
  - q/k projections produce qhT/khT [o, s] (o on partitions, stacked in
    head-pairs of 128); v projection produces vh natural [s, o]
  - scores computed transposed, sT[j, i] = khT.T @ qhT, two heads packed
    on the 128x128 PE array via row tiling (K=64 each)
  - softmax: exp on ACT (scale=1/8 folded in, no max subtraction needed
    since scores ~ N(0,1)); denominator via an appended ones-column in
    the attnV matmul (lhsT = [vh | 1] -> out rows 0..63 = outT, row 64 =
    rowsum); normalization folded in before the out projection
  - all matmul operands float32r (full PE rate, ~1.9e-4 matmul rel err)
"""
import numpy as np

B, S, D = 2, 2048, 1024
HEADS, DK = 16, 64
NCORES, DP, TP = 8, 2, 4
OPC = D // TP          # 256 output dims per core
HPC = HEADS // TP      # 4 heads per core
NDC = D // 128         # 8 contraction chunks
NST = S // 128         # 16 s-tiles
NSC = S // 512         # 4 s-chunks

_cache = {}


def _build():
    import concourse.mybir as mybir
    import concourse.tile as tile
    from concourse import bacc

    F32 = mybir.dt.float32
    F32R = mybir.dt.float32r
    BF16 = mybir.dt.bfloat16
    Exp = mybir.ActivationFunctionType.Exp
    Copy = mybir.ActivationFunctionType.Copy
    Ident = mybir.ActivationFunctionType.Identity

    nc = bacc.Bacc("TRN2", target_bir_lowering=False, debug=False)

    xq_d = nc.dram_tensor("xqt", [D, S], BF16, kind="ExternalInput")
    xk_d = nc.dram_tensor("xkt", [D, S], BF16, kind="ExternalInput")
    xv_d = nc.dram_tensor("xvt", [D, S], BF16, kind="ExternalInput")
    wq_d = nc.dram_tensor("wqt", [D, OPC], BF16, kind="ExternalInput")
    wk_d = nc.dram_tensor("wkt", [D, OPC], BF16, kind="ExternalInput")
    wv_d = nc.dram_tensor("wvt", [D, OPC], BF16, kind="ExternalInput")
    bq_d = nc.dram_tensor("bq", [2, 128, 1], F32, kind="ExternalInput")
    bk_d = nc.dram_tensor("bk", [2, 128, 1], F32, kind="ExternalInput")
    bv_d = nc.dram_tensor("bv", [1, OPC], BF16, kind="ExternalInput")
    wo_d = nc.dram_tensor("wot", [2, 128, D], F32R, kind="ExternalInput")
    out_d = nc.dram_tensor("out", [S, D], F32, kind="ExternalOutput")

    with tile.TileContext(nc) as tc:
        from contextlib import ExitStack
        es = ExitStack()
        with es:
            wp = es.enter_context(tc.tile_pool(name="wp", bufs=1))
            acts = es.enter_context(tc.tile_pool(name="acts", bufs=1))

            # ---- phase 0: weights, biases, constants
            wq_t = [wp.tile([128, OPC], BF16, name=f"wq{i}") for i in range(NDC)]
            wk_t = [wp.tile([128, OPC], BF16, name=f"wk{i}") for i in range(NDC)]
            wv_t = [wp.tile([128, OPC], BF16, name=f"wv{i}") for i in range(NDC)]
            for i in range(NDC):
                nc.sync.dma_start(wq_t[i][:], wq_d.ap()[i * 128:(i + 1) * 128, :])
                nc.sync.dma_start(wk_t[i][:], wk_d.ap()[i * 128:(i + 1) * 128, :])
                nc.sync.dma_start(wv_t[i][:], wv_d.ap()[i * 128:(i + 1) * 128, :])
            wo_t = [wp.tile([128, D], F32R, name=f"wo{h}") for h in range(2)]
            bq_t = [wp.tile([128, 1], F32, name=f"bq{h}") for h in range(2)]
            bk_t = [wp.tile([128, 1], F32, name=f"bk{h}") for h in range(2)]
            for h in range(2):
                nc.sync.dma_start(wo_t[h][:], wo_d.ap()[h])
                nc.sync.dma_start(bq_t[h][:], bq_d.ap()[h])
                nc.sync.dma_start(bk_t[h][:], bk_d.ap()[h])
            bv_r = wp.tile([1, OPC], BF16, name="bv_r")
            nc.sync.dma_start(bv_r[:], bv_d.ap())

            ones_f = wp.tile([128, 1], F32, name="ones_f")
            nc.vector.memset(ones_f[:], 1.0)
            onesf1 = wp.tile([1, 128], F32, name="onesf1")
            nc.vector.memset(onesf1[:], 1.0)
            ones1 = wp.tile([1, 128], F32R, name="ones1")
            nc.vector.tensor_copy(ones1[:], onesf1[:])
            ones1h = wp.tile([1, 128], BF16, name="ones1h")
            nc.vector.tensor_copy(ones1h[:], onesf1[:])

            # bv broadcast to all 128 partitions: bv2 = ones1.T @ bv_r
            with tc.tile_pool(name="bv_ps", bufs=1, space="PSUM") as bvps:
                pbv = bvps.tile([128, OPC], F32, name="pbv")
                nc.tensor.matmul(pbv[:], ones1h[:], bv_r[:], start=True, stop=True)
                bv2 = wp.tile([128, OPC], F32, name="bv2")
                nc.vector.tensor_copy(bv2[:], pbv[:])

            # persistent activations
            qh_st = [acts.tile([128, S], F32R, name=f"qh{h}") for h in range(2)]
            kh_st = [acts.tile([128, S], F32R, name=f"kh{h}") for h in range(2)]
            vh_aug = [[acts.tile([128, DK + 1], F32R, name=f"va{h}_{j}")
                       for j in range(NST)] for h in range(HPC)]
            stacked = [acts.tile([128, S], F32R, name=f"st{h}") for h in range(2)]

            # ---- phase 1: projections
            with tc.tile_pool(name="xin", bufs=1) as xp, \
                 tc.tile_pool(name="pps", bufs=4, space="PSUM") as pps, \
                 tc.tile_pool(name="vps", bufs=4, space="PSUM") as vps:
                for xd, wt, dest in ((xq_d, wq_t, qh_st), (xk_d, wk_t, kh_st)):
                    bias = bq_t if dest is qh_st else bk_t
                    for sc in range(NSC):
                        xt = [xp.tile([128, 512], BF16, name="xt", tag="xt", bufs=18)
                              for _ in range(NDC)]
                        for dc in range(NDC):
                            nc.sync.dma_start(
                                xt[dc][:], xd.ap()[dc * 128:(dc + 1) * 128,
                                                   sc * 512:(sc + 1) * 512])
                        for hp in range(2):
                            p = pps.tile([128, 512], F32, name="pp", tag="pp")
                            for dc in range(NDC):
                                nc.tensor.matmul(
                                    p[:], wt[dc][:, hp * 128:(hp + 1) * 128],
                                    xt[dc][:], start=(dc == 0), stop=(dc == NDC - 1))
                            nc.scalar.activation(
                                dest[hp][:, sc * 512:(sc + 1) * 512], p[:],
                                Ident, bias=bias[hp][:])
                # v projection: natural layout, x as stationary
                for sc in range(NSC):
                    xt = [xp.tile([128, 512], BF16, name="xt", tag="xt", bufs=18)
                          for _ in range(NDC)]
                    for dc in range(NDC):
                        nc.sync.dma_start(
                            xt[dc][:], xv_d.ap()[dc * 128:(dc + 1) * 128,
                                                 sc * 512:(sc + 1) * 512])
                    for st4 in range(4):
                        st = sc * 4 + st4
                        pv = vps.tile([128, OPC], F32, name="pv", tag="pv")
                        for dc in range(NDC):
                            nc.tensor.matmul(
                                pv[:], xt[dc][:, st4 * 128:(st4 + 1) * 128],
                                wv_t[dc][:], start=(dc == 0), stop=(dc == NDC - 1))
                        for h in range(HPC):
                            nc.vector.tensor_add(
                                vh_aug[h][st][:, 0:DK], pv[:, h * DK:(h + 1) * DK],
                                bv2[:, h * DK:(h + 1) * DK])
                            nc.vector.tensor_copy(vh_aug[h][st][:, DK:DK + 1],
                                                  ones_f[:])

            # ---- phase 2: attention per head-pair
            with tc.tile_pool(name="sps", bufs=2, space="PSUM") as sps, \
                 tc.tile_pool(name="avps", bufs=3, space="PSUM") as avps, \
                 tc.tile_pool(name="r2ps", bufs=1, space="PSUM") as r2ps, \
                 tc.tile_pool(name="ep", bufs=4) as ep, \
                 tc.tile_pool(name="rp", bufs=4) as rp:
                for hp in range(2):
                    for ic in range(NSC):
                        av = [avps.tile([128, 512], F32, name="av", tag="av")
                              for _ in range(2)]
                        for j in range(NST):
                            sp = sps.tile([128, 1024], F32, name="sp", tag="sp")
                            nc.tensor.matmul(
                                sp[:, 0:512], kh_st[hp][0:64, j * 128:(j + 1) * 128],
                                qh_st[hp][0:64, ic * 512:(ic + 1) * 512],
                                start=True, stop=True, tile_position=(0, 0))
                            nc.tensor.matmul(
                                sp[:, 512:1024], kh_st[hp][64:128, j * 128:(j + 1) * 128],
                                qh_st[hp][64:128, ic * 512:(ic + 1) * 512],
                                start=True, stop=True, tile_position=(64, 0))
                            et = ep.tile([128, 1024], F32R, name="et", tag="et")
                            nc.scalar.activation(et[:], sp[:], Exp, scale=0.125)
                            for h2 in range(2):
                                nc.tensor.matmul(
                                    av[h2][0:DK + 1, :], vh_aug[hp * 2 + h2][j][:],
                                    et[:, h2 * 512:(h2 + 1) * 512],
                                    start=(j == 0), stop=(j == NST - 1),
                                    skip_group_check=True)
                        for h2 in range(2):
                            dnm = rp.tile([1, 512], F32, name="dnm", tag="dnm")
                            nc.vector.tensor_copy(dnm[:], av[h2][DK:DK + 1, :])
                            rcf = rp.tile([1, 512], F32, name="rcf", tag="rcf")
                            nc.vector.reciprocal_approx_fast(rcf[:], dnm[:])
                            rci = rp.tile([1, 512], F32R, name="rci", tag="rci")
                            nc.vector.tensor_copy(rci[:], rcf[:])
                            r2 = r2ps.tile([64, 512], F32, name="r2", tag="r2")
                            nc.tensor.matmul(r2[:], ones1[0:1, 0:64], rci[:],
                                             start=True, stop=True)
                            r2s = rp.tile([64, 512], F32, name="r2s", tag="r2s")
                            nc.scalar.activation(r2s[:], r2[:], Copy)
                            nc.vector.tensor_mul(
                                stacked[hp][h2 * 64:(h2 + 1) * 64,
                                            ic * 512:(ic + 1) * 512],
                                av[h2][0:DK, :], r2s[:])

            # ---- phase 3: output projection (partial; host adds across cores)
            with tc.tile_pool(name="ops", bufs=4, space="PSUM") as ops, \
                 tc.tile_pool(name="obp", bufs=4) as obp:
                for it in range(NST):
                    for mc in range(2):
                        po = ops.tile([128, 512], F32, name="po", tag="po")
                        for hp in range(2):
                            nc.tensor.matmul(
                                po[:], stacked[hp][:, it * 128:(it + 1) * 128],
                                wo_t[hp][:, mc * 512:(mc + 1) * 512],
                                start=(hp == 0), stop=(hp == 1))
                        ot = obp.tile([128, 512], F32, name="ot", tag="ot")
                        nc.vector.tensor_copy(ot[:], po[:])
                        nc.sync.dma_start(
                            out_d.ap()[it * 128:(it + 1) * 128,
                                       mc * 512:(mc + 1) * 512], ot[:])

    nc.compile()
    return nc


def _prep_inputs(q, k, v, Wq, bq, Wk, bk, Wv, bv, Wo, bo):
    import ml_dtypes
    f = np.float32
    bf = ml_dtypes.bfloat16
    xT = {}
    for g in range(DP):
        xT[("q", g)] = np.ascontiguousarray(np.asarray(q[g], f).T.astype(bf))
        xT[("k", g)] = np.ascontiguousarray(np.asarray(k[g], f).T.astype(bf))
        xT[("v", g)] = np.ascontiguousarray(np.asarray(v[g], f).T.astype(bf))
    Wq, Wk, Wv, Wo = (np.asarray(a, f) for a in (Wq, Wk, Wv, Wo))
    bq, bk, bv = (np.asarray(a, f) for a in (bq, bk, bv))
    in_maps = []
    for c in range(NCORES):
        g, r = divmod(c, TP)
        sl = slice(r * OPC, (r + 1) * OPC)
        in_maps.append({
            "xqt": xT[("q", g)], "xkt": xT[("k", g)], "xvt": xT[("v", g)],
            "wqt": np.ascontiguousarray(Wq[sl].T.astype(bf)),
            "wkt": np.ascontiguousarray(Wk[sl].T.astype(bf)),
            "wvt": np.ascontiguousarray(Wv[sl].T.astype(bf)),
            "bq": bq[sl].reshape(2, 128, 1),
            "bk": bk[sl].reshape(2, 128, 1),
            "bv": bv[sl].reshape(1, OPC).astype(bf),
            "wot": np.ascontiguousarray(Wo[:, sl].T).reshape(2, 128, D),
        })
    return in_maps


def kernel(q, k, v, Wq, bq, Wk, bk, Wv, bv, Wo, bo, _trace=False):
    from concourse.bass_utils import run_bass_kernel_spmd

    if "nc" not in _cache:
        _cache["nc"] = _build()
    nc = _cache["nc"]
    in_maps = _prep_inputs(q, k, v, Wq, bq, Wk, bk, Wv, bv, Wo, bo)
    res = run_bass_kernel_spmd(nc, in_maps, list(range(NCORES)), trace=_trace)
    _cache["last_exec_time_ns"] = res.exec_time_ns
    _cache["last_res"] = res
    parts = [res.results[c]["out"] for c in range(NCORES)]
    bo = np.asarray(bo, np.float32)
    out = np.empty((B, S, D), np.float32)
    for g in range(DP):
        acc = parts[g * TP].astype(np.float32)
        for r in range(1, TP):
            acc = acc + parts[g * TP + r]
        out[g] = acc + bo
    return out
